# revision 30
# baseline (speedup 1.0000x reference)
"""Trainium2 Bass kernel for nn_CorrelationHead (8-core SPMD, data parallel over B).

Math reformulation (same as v0, ~1e-6 vs the jax reference in fp32):
  corr[b,p,q,i,j] = sum_c patch1[b,c,i,j] * patch2[b,c, i+2p-20, j+2q-20]
  out[b,n] = sum_{ij,yx} A[b][ij,yx] * W3[n,ij,yx] + bias[n]
  where A[b] = P1[b]^T P2[b] ([ij,yx], contracted over c) and W3 gathers
  w_bbox onto the 49x49 grid (out-of-patch displacements drop out).

v1 layout (vs the v0 baseline at ~28.8us):
  - host packs X[b] = [128ch, 49 p1 cols (ij parity-sorted) | 49 p2 cols]
    bf16; FOUR chunked HWDGE DMAs (24/16/16/8 samples) all on the sync
    queue (FIFO ring -> chunks complete in order, ~3136B rows), aux
    weights on the scalar queue. No SWDGE.
  - PE warm-up: 6 dummy N=512 matmuls over a zeroed scratch during the
    DMA wait, so the HAM clock-gate reaches 2.4GHz before real work.
  - stage 1: ONE K=128 matmul per sample (LDW cost ~ stationary COLS=49,
    not K): out PSUM A^T[yx=49, ij=49], 8 samples per PSUM bank.
  - casts: per 8-sample group, even-ij cols -> ACT copy into
    acat[0:49, b, c], odd-ij cols -> DVE copy into acat[64:113, b, c]
    (quadrant-aligned partition shift) -> stage-2 K folds two ij per
    chunk (25 chunks instead of 49).
  - stage 2: W-stationary accumulating matmuls: lhsT = w2s[0:K, 4c:4c+4]
    (LDW = 4 cols ~ free), rhs = acat[0:K, :, c] (N=64 samples), 25
    chunks accumulate into PSUM [4, 64]; bias folded via a ones row at
    partition 113 on chunk 0. Output copied once and DMA'd - no chain
    merge, no selector matmul.
"""

import os

import numpy as np

import concourse.bass as bass
import concourse.mybir as mybir
from concourse import bacc
from concourse.bass_utils import run_bass_kernel_spmd

_BISECT = set(os.environ.get("KBISECT", "").split(","))

N_CORES = 8
B, C, HW = 512, 128, 49
BS = B // N_CORES          # 64 samples per core
NG = 8                     # cast groups
GS = BS // NG              # 8 samples per group
FW = 2 * HW                # 98 packed cols per sample: p1-sorted | p2
NE = 25                    # even-ij count (ij % 2 == 0)
NO = 24                    # odd-ij count
CHUNKS = (16, 16, 16, 16)  # samples per input DMA (sum = 64)
NDUMMY = 6                 # PE warm-up matmuls (N=512 each)

_F32 = mybir.dt.float32
_BF16 = mybir.dt.bfloat16


def _build_w3(w_bbox: np.ndarray) -> np.ndarray:
    """W3[n, ij, yx] such that out[b,n] = sum W3[n,ij,yx] * A[b,ij,yx]."""
    W3 = np.zeros((4, 49, 49), np.float32)
    for i in range(7):
        for j in range(7):
            for y in range(7):
                for x in range(7):
                    if (y - i) % 2 == 0 and (x - j) % 2 == 0:
                        p = (y - i + 20) // 2
                        q = (x - j + 20) // 2
                        W3[:, i * 7 + j, y * 7 + x] = w_bbox[
                            :, ((p * 21 + q) * 7 + i) * 7 + j
                        ]
    return W3


def build_nc() -> bass.Bass:
    nc = bacc.Bacc("TRN2", target_bir_lowering=False, debug=False)
    xd = nc.dram_tensor("xd", [C, BS, FW], _BF16, kind="ExternalInput")
    w2d = nc.dram_tensor("w2d", [C, 4 * HW], _BF16, kind="ExternalInput")
    zd = nc.dram_tensor("zd", [15, BS, NE], _BF16, kind="ExternalInput")
    outd = nc.dram_tensor("out", [100, BS], _F32, kind="ExternalOutput")

    from contextlib import ExitStack

    with ExitStack() as ctx:
        xs = ctx.enter_context(nc.sbuf_tensor("xs", [C, BS, FW], _BF16))
        acat = ctx.enter_context(nc.sbuf_tensor("acat", [C, BS, NE], _BF16))
        acato = (
            ctx.enter_context(nc.sbuf_tensor("acato", [C, BS, NO], _BF16))
            if "noshift" in _BISECT
            else None
        )
        w2s = ctx.enter_context(nc.sbuf_tensor("w2s", [C, 4 * HW], _BF16))
        scr = ctx.enter_context(nc.sbuf_tensor("scr", [C, 512], _BF16))
        out_sb = ctx.enter_context(nc.sbuf_tensor("out_sb", [100, BS], _F32))
        # [partition, bank, sample-slot, col]: 8 banks x 8 slots x 64 f32
        ps = ctx.enter_context(nc.psum_tensor("ps", [128, 8, 8, 64], _F32))

        sD = [ctx.enter_context(nc.semaphore(f"sD{i}")) for i in range(4)]
        (sAux, sScr, sZ, sS1, sCastE, sCastD, sS2, sOut, sDone) = (
            ctx.enter_context(nc.semaphore(nm))
            for nm in (
                "sAux", "sScr", "sZ", "sS1", "sCastE", "sCastD",
                "sS2", "sOut", "sDone",
            )
        )
        block = ctx.enter_context(nc.Block())

        # chunk boundaries in samples
        bounds = np.cumsum((0,) + CHUNKS)
        chunk_of_group = {}
        for k in range(4):
            g0 = bounds[k] // GS
            chunk_of_group[g0] = k

        @block.sync
        def _(sync):
            for k in range(4):
                lo, hi = int(bounds[k]), int(bounds[k + 1])
                sync.dma_start(
                    out=xs[:, lo:hi, :], in_=xd[:, lo:hi, :]
                ).then_inc(sD[k], 16)
            sync.wait_ge(sOut, 1)
            sync.dma_start(out=outd[:], in_=out_sb[:]).then_inc(sDone, 16)
            sync.wait_ge(sDone, 16)

        @block.scalar
        def _(scalar):
            scalar.dma_start(out=w2s[:], in_=w2d[:]).then_inc(sAux, 16)
            # rows 49-63 of acat: zeros + the bias ones-column (row 63),
            # delivered by DMA instead of partial-partition memsets.
            scalar.dma_start(out=acat[49:64, :, :], in_=zd[:]).then_inc(
                sZ, 16
            )
            for g in range(NG):
                scalar.wait_ge(sS1, g + 1)
                nc.scalar.copy(
                    acat[0:49, g * GS : (g + 1) * GS, 0:NE],
                    ps[0:49, g % 6, 0:GS, 0:NE],
                ).then_inc(sCastE, 1)

        @block.vector
        def _(vector):
            # zero the stage-2 bank once so the [100, BS] output copy reads
            # defined data between the four strip accumulators.
            nc.vector.memset(ps[0:128, 6, :, :], 0.0)
            odst = acato if "noshift" in _BISECT else acat
            olo = 0 if "noshift" in _BISECT else 64
            for g in range(NG):
                # ACT and DVE must never read the SAME psum bank
                # concurrently (HW hazard, probe-verified): trail the
                # even-ij (ACT) cast of this group by one step.
                vector.wait_ge(sCastE, g + 1)
                nc.vector.tensor_copy(
                    odst[olo : olo + 49, g * GS : (g + 1) * GS, 0:NO],
                    ps[0:49, g % 6, 0:GS, NE:HW],
                ).then_inc(sCastD, 1)
            vector.wait_ge(sS2, 1)
            nc.vector.tensor_copy(out_sb[:], ps[0:100, 6, 0, 0:BS]).then_inc(
                sOut, 1
            )

        @block.tensor
        def _(tensor):
            for g in range(NG):
                if g in chunk_of_group:
                    tensor.wait_ge(sD[chunk_of_group[g]], 16)
                if g >= 6:
                    tensor.wait_ge(sCastE, g - 5)
                    tensor.wait_ge(sCastD, g - 5)
                for k in range(GS):
                    s = g * GS + k
                    mm = nc.tensor.matmul(
                        ps[0:49, g % 6, k, 0:49],
                        xs[:, s, 49:98],
                        xs[:, s, 0:49],
                        start=True,
                        stop=True,
                    )
                    if k == GS - 1:
                        mm.then_inc(sS1, 1)
            tensor.wait_ge(sAux, 16)
            tensor.wait_ge(sZ, 16)
            tensor.wait_ge(sCastE, NG)
            tensor.wait_ge(sCastD, NG)
            if "noshift" in _BISECT:
                for c in range(HW):
                    if c < NE:
                        kc = 64 if c == 0 else 49
                        rhs = acat[0:kc, :, c]
                    else:
                        kc = 49
                        rhs = acato[0:kc, :, c - NE]
                    mm = nc.tensor.matmul(
                        ps[0:4, 6, 0, 0:BS],
                        w2s[0:kc, 4 * c : 4 * c + 4],
                        rhs,
                        start=(c == 0),
                        stop=(c == HW - 1),
                    )
            else:
                for c in range(NE):
                    kc = 113 if c < NO else 49
                    s = c % 4
                    mm = nc.tensor.matmul(
                        ps[32 * s : 32 * s + 4, 6, 0, 0:BS],
                        w2s[0:kc, 4 * c : 4 * c + 4],
                        acat[0:kc, :, c],
                        start=(c < 4),
                        stop=(c >= NE - 4),
                        tile_position=(0, 32 * s),
                    )
            mm.then_inc(sS2, 1)

    nc.compile()
    return nc


def _prep_inputs(inputs):
    import ml_dtypes

    bf = ml_dtypes.bfloat16
    p1 = np.asarray(inputs["patch1"], np.float32).reshape(B, C, HW)
    p2 = np.asarray(inputs["patch2"], np.float32).reshape(B, C, HW)
    order = [ij for ij in range(HW) if ij % 2 == 0] + [
        ij for ij in range(HW) if ij % 2 == 1
    ]
    X = np.empty((B, C, FW), np.float32)
    X[:, :, 0:HW] = p1[:, :, order]
    X[:, :, HW:FW] = p2
    Xb = X.astype(bf)

    W3 = _build_w3(np.asarray(inputs["w_bbox"], np.float32))
    bias = np.asarray(inputs["b_bbox"], np.float32)
    ije = order[0:NE]
    ijo = order[NE:HW]
    w2 = np.zeros((C, HW, 4), np.float32)
    if "noshift" in _BISECT:
        for c in range(NE):
            w2[0:49, c, :] = W3[:, ije[c], :].T
        for c in range(NO):
            w2[0:49, NE + c, :] = W3[:, ijo[c], :].T
    else:
        for c in range(NE):
            w2[0:49, c, :] = W3[:, ije[c], :].T
        for c in range(NO):
            w2[64:113, c, :] = W3[:, ijo[c], :].T
    w2[63, 0, :] = bias
    w2b = np.ascontiguousarray(w2.reshape(C, 4 * HW)).astype(bf)
    # acat rows 49-63 payload: zeros, with the chunk-0 ones-column at
    # row 63 pairing with the bias row of w2s.
    z = np.zeros((15, BS, NE), np.float32)
    z[14, :, 0] = 1.0
    zb = z.astype(bf)

    in_maps = []
    for core in range(N_CORES):
        sl = slice(core * BS, (core + 1) * BS)
        xc = np.ascontiguousarray(Xb[sl].transpose(1, 0, 2))
        in_maps.append({"xd": xc, "w2d": w2b, "zd": zb})
    return in_maps


def _run(inputs, trace: bool = False):
    nc = build_nc()
    in_maps = _prep_inputs(inputs)
    res = run_bass_kernel_spmd(
        nc, in_maps, core_ids=list(range(N_CORES)), trace=trace
    )
    outs = []
    for c in range(N_CORES):
        o = res.results[c]["out"]
        outs.append((o[0:4] + o[32:36] + o[64:68] + o[96:100]).T)
    out = np.concatenate(outs, axis=0).astype(np.float32)
    return out, res


def kernel(**inputs) -> np.ndarray:
    out, _ = _run(inputs, trace=False)
    return out


# revision 31
# speedup vs baseline: 1.0064x; 1.0064x over previous
"""Trainium2 Bass kernel for nn_CorrelationHead (8-core SPMD, data parallel over B).

Math reformulation (same as v0, ~1e-6 vs the jax reference in fp32):
  corr[b,p,q,i,j] = sum_c patch1[b,c,i,j] * patch2[b,c, i+2p-20, j+2q-20]
  out[b,n] = sum_{ij,yx} A[b][ij,yx] * W3[n,ij,yx] + bias[n]
  where A[b] = P1[b]^T P2[b] ([ij,yx], contracted over c) and W3 gathers
  w_bbox onto the 49x49 grid (out-of-patch displacements drop out).

v1 layout (vs the v0 baseline at ~28.8us):
  - host packs X[b] = [128ch, 49 p1 cols (ij parity-sorted) | 49 p2 cols]
    bf16; FOUR chunked HWDGE DMAs (24/16/16/8 samples) all on the sync
    queue (FIFO ring -> chunks complete in order, ~3136B rows), aux
    weights on the scalar queue. No SWDGE.
  - PE warm-up: 6 dummy N=512 matmuls over a zeroed scratch during the
    DMA wait, so the HAM clock-gate reaches 2.4GHz before real work.
  - stage 1: ONE K=128 matmul per sample (LDW cost ~ stationary COLS=49,
    not K): out PSUM A^T[yx=49, ij=49], 8 samples per PSUM bank.
  - casts: per 8-sample group, even-ij cols -> ACT copy into
    acat[0:49, b, c], odd-ij cols -> DVE copy into acat[64:113, b, c]
    (quadrant-aligned partition shift) -> stage-2 K folds two ij per
    chunk (25 chunks instead of 49).
  - stage 2: W-stationary accumulating matmuls: lhsT = w2s[0:K, 4c:4c+4]
    (LDW = 4 cols ~ free), rhs = acat[0:K, :, c] (N=64 samples), 25
    chunks accumulate into PSUM [4, 64]; bias folded via a ones row at
    partition 113 on chunk 0. Output copied once and DMA'd - no chain
    merge, no selector matmul.
"""

import os

import numpy as np

import concourse.bass as bass
import concourse.mybir as mybir
from concourse import bacc
from concourse.bass_utils import run_bass_kernel_spmd

_BISECT = set(os.environ.get("KBISECT", "").split(","))

N_CORES = 8
B, C, HW = 512, 128, 49
BS = B // N_CORES          # 64 samples per core
NG = 8                     # cast groups
GS = BS // NG              # 8 samples per group
FW = 2 * HW                # 98 packed cols per sample: p1-sorted | p2
NE = 25                    # even-ij count (ij % 2 == 0)
NO = 24                    # odd-ij count
CHUNKS = (24, 16, 16, 8)   # samples per input DMA (sum = 64)
NDUMMY = 6                 # PE warm-up matmuls (N=512 each)

_F32 = mybir.dt.float32
_BF16 = mybir.dt.bfloat16


def _build_w3(w_bbox: np.ndarray) -> np.ndarray:
    """W3[n, ij, yx] such that out[b,n] = sum W3[n,ij,yx] * A[b,ij,yx]."""
    W3 = np.zeros((4, 49, 49), np.float32)
    for i in range(7):
        for j in range(7):
            for y in range(7):
                for x in range(7):
                    if (y - i) % 2 == 0 and (x - j) % 2 == 0:
                        p = (y - i + 20) // 2
                        q = (x - j + 20) // 2
                        W3[:, i * 7 + j, y * 7 + x] = w_bbox[
                            :, ((p * 21 + q) * 7 + i) * 7 + j
                        ]
    return W3


def build_nc() -> bass.Bass:
    nc = bacc.Bacc("TRN2", target_bir_lowering=False, debug=False)
    xd = nc.dram_tensor("xd", [C, BS, FW], _BF16, kind="ExternalInput")
    w2d = nc.dram_tensor("w2d", [C, 4 * HW], _BF16, kind="ExternalInput")
    zd = nc.dram_tensor("zd", [15, BS, NE], _BF16, kind="ExternalInput")
    outd = nc.dram_tensor("out", [100, BS], _F32, kind="ExternalOutput")

    from contextlib import ExitStack

    with ExitStack() as ctx:
        xs = ctx.enter_context(nc.sbuf_tensor("xs", [C, BS, FW], _BF16))
        acat = ctx.enter_context(nc.sbuf_tensor("acat", [C, BS, NE], _BF16))
        acato = (
            ctx.enter_context(nc.sbuf_tensor("acato", [C, BS, NO], _BF16))
            if "noshift" in _BISECT
            else None
        )
        w2s = ctx.enter_context(nc.sbuf_tensor("w2s", [C, 4 * HW], _BF16))
        scr = ctx.enter_context(nc.sbuf_tensor("scr", [C, 512], _BF16))
        out_sb = ctx.enter_context(nc.sbuf_tensor("out_sb", [100, BS], _F32))
        # [partition, bank, sample-slot, col]: 8 banks x 8 slots x 64 f32
        ps = ctx.enter_context(nc.psum_tensor("ps", [128, 8, 8, 64], _F32))

        sD = [ctx.enter_context(nc.semaphore(f"sD{i}")) for i in range(4)]
        (sAux, sScr, sZ, sS1, sCastE, sCastD, sS2, sOut, sDone) = (
            ctx.enter_context(nc.semaphore(nm))
            for nm in (
                "sAux", "sScr", "sZ", "sS1", "sCastE", "sCastD",
                "sS2", "sOut", "sDone",
            )
        )
        block = ctx.enter_context(nc.Block())

        # chunk boundaries in samples
        bounds = np.cumsum((0,) + CHUNKS)
        chunk_of_group = {}
        for k in range(4):
            g0 = bounds[k] // GS
            chunk_of_group[g0] = k

        @block.sync
        def _(sync):
            for k in range(4):
                lo, hi = int(bounds[k]), int(bounds[k + 1])
                sync.dma_start(
                    out=xs[:, lo:hi, :], in_=xd[:, lo:hi, :]
                ).then_inc(sD[k], 16)
            sync.wait_ge(sOut, 1)
            sync.dma_start(out=outd[:], in_=out_sb[:]).then_inc(sDone, 16)
            sync.wait_ge(sDone, 16)

        @block.scalar
        def _(scalar):
            scalar.dma_start(out=w2s[:], in_=w2d[:]).then_inc(sAux, 16)
            # rows 49-63 of acat: zeros + the bias ones-column (row 63),
            # delivered by DMA instead of partial-partition memsets.
            scalar.dma_start(out=acat[49:64, :, :], in_=zd[:]).then_inc(
                sZ, 16
            )
            scalar.wait_ge(sZ, 16)
            for g in range(NG):
                scalar.wait_ge(sS1, g + 1)
                nc.scalar.copy(
                    acat[0:49, g * GS : (g + 1) * GS, 0:NE],
                    ps[0:49, g % 6, 0:GS, 0:NE],
                ).then_inc(sCastE, 1)

        @block.vector
        def _(vector):
            # zero the stage-2 bank once so the [100, BS] output copy reads
            # defined data between the four strip accumulators.
            nc.vector.memset(ps[0:128, 6, :, :], 0.0)
            odst = acato if "noshift" in _BISECT else acat
            olo = 0 if "noshift" in _BISECT else 64
            for g in range(NG):
                # ACT and DVE must never read the SAME psum bank
                # concurrently (HW hazard, probe-verified): trail the
                # even-ij (ACT) cast of this group by one step.
                vector.wait_ge(sCastE, g + 1)
                nc.vector.tensor_copy(
                    odst[olo : olo + 49, g * GS : (g + 1) * GS, 0:NO],
                    ps[0:49, g % 6, 0:GS, NE:HW],
                ).then_inc(sCastD, 1)
            vector.wait_ge(sS2, 1)
            nc.vector.tensor_copy(out_sb[:], ps[0:100, 6, 0, 0:BS]).then_inc(
                sOut, 1
            )

        @block.tensor
        def _(tensor):
            for g in range(NG):
                if g in chunk_of_group:
                    tensor.wait_ge(sD[chunk_of_group[g]], 16)
                if g >= 6:
                    tensor.wait_ge(sCastE, g - 5)
                    tensor.wait_ge(sCastD, g - 5)
                for k in range(GS):
                    s = g * GS + k
                    mm = nc.tensor.matmul(
                        ps[0:49, g % 6, k, 0:49],
                        xs[:, s, 49:98],
                        xs[:, s, 0:49],
                        start=True,
                        stop=True,
                    )
                    if k == GS - 1:
                        mm.then_inc(sS1, 1)
            tensor.wait_ge(sAux, 16)
            tensor.wait_ge(sZ, 16)
            tensor.wait_ge(sCastE, NG)
            tensor.wait_ge(sCastD, NG)
            if "noshift" in _BISECT:
                for c in range(HW):
                    if c < NE:
                        kc = 64 if c == 0 else 49
                        rhs = acat[0:kc, :, c]
                    else:
                        kc = 49
                        rhs = acato[0:kc, :, c - NE]
                    mm = nc.tensor.matmul(
                        ps[0:4, 6, 0, 0:BS],
                        w2s[0:kc, 4 * c : 4 * c + 4],
                        rhs,
                        start=(c == 0),
                        stop=(c == HW - 1),
                    )
            else:
                for c in range(NE):
                    kc = 113 if c < NO else 49
                    s = c % 4
                    mm = nc.tensor.matmul(
                        ps[32 * s : 32 * s + 4, 6, 0, 0:BS],
                        w2s[0:kc, 4 * c : 4 * c + 4],
                        acat[0:kc, :, c],
                        start=(c < 4),
                        stop=(c >= NE - 4),
                        tile_position=(0, 32 * s),
                    )
            mm.then_inc(sS2, 1)

    nc.compile()
    return nc


def _prep_inputs(inputs):
    import ml_dtypes

    bf = ml_dtypes.bfloat16
    p1 = np.asarray(inputs["patch1"], np.float32).reshape(B, C, HW)
    p2 = np.asarray(inputs["patch2"], np.float32).reshape(B, C, HW)
    order = [ij for ij in range(HW) if ij % 2 == 0] + [
        ij for ij in range(HW) if ij % 2 == 1
    ]
    X = np.empty((B, C, FW), np.float32)
    X[:, :, 0:HW] = p1[:, :, order]
    X[:, :, HW:FW] = p2
    Xb = X.astype(bf)

    W3 = _build_w3(np.asarray(inputs["w_bbox"], np.float32))
    bias = np.asarray(inputs["b_bbox"], np.float32)
    ije = order[0:NE]
    ijo = order[NE:HW]
    w2 = np.zeros((C, HW, 4), np.float32)
    if "noshift" in _BISECT:
        for c in range(NE):
            w2[0:49, c, :] = W3[:, ije[c], :].T
        for c in range(NO):
            w2[0:49, NE + c, :] = W3[:, ijo[c], :].T
    else:
        for c in range(NE):
            w2[0:49, c, :] = W3[:, ije[c], :].T
        for c in range(NO):
            w2[64:113, c, :] = W3[:, ijo[c], :].T
    w2[63, 0, :] = bias
    w2b = np.ascontiguousarray(w2.reshape(C, 4 * HW)).astype(bf)
    # acat rows 49-63 payload: zeros, with the chunk-0 ones-column at
    # row 63 pairing with the bias row of w2s.
    z = np.zeros((15, BS, NE), np.float32)
    z[14, :, 0] = 1.0
    zb = z.astype(bf)

    in_maps = []
    for core in range(N_CORES):
        sl = slice(core * BS, (core + 1) * BS)
        xc = np.ascontiguousarray(Xb[sl].transpose(1, 0, 2))
        in_maps.append({"xd": xc, "w2d": w2b, "zd": zb})
    return in_maps


def _run(inputs, trace: bool = False):
    nc = build_nc()
    in_maps = _prep_inputs(inputs)
    res = run_bass_kernel_spmd(
        nc, in_maps, core_ids=list(range(N_CORES)), trace=trace
    )
    outs = []
    for c in range(N_CORES):
        o = res.results[c]["out"]
        outs.append((o[0:4] + o[32:36] + o[64:68] + o[96:100]).T)
    out = np.concatenate(outs, axis=0).astype(np.float32)
    return out, res


def kernel(**inputs) -> np.ndarray:
    out, _ = _run(inputs, trace=False)
    return out
